# revision 4
# baseline (speedup 1.0000x reference)
"""AttentionPooling (segment_reduce) Trainium2 kernel.

att = sigmoid([input_rep, final_rep] @ W_lin.T + b_lin)
g   = att * (final_rep @ W_last.T + b_last)
out = segment_sum(g, graph_index, 16384)          # graph_index sorted

Strategy (8 NeuronCores, pure data-parallel, no collectives):
  graph_index is sorted, so a contiguous node range covers a contiguous
  graph range.  Host greedily packs whole graphs into "windows" of
  <= WIN_NODES nodes spanning <= 128 graphs; ~137 windows cover all 500k
  nodes = 8 cores x 17 windows.  Each core gets its windows as a padded
  node stream in feature-major fp8 layout.

All matmuls run as fp8e4m3 DoubleRow (2 contraction chunks per mm,
0.5 PE cycles per output column).  Accuracy is recovered with residual
streams: per node the five fp8 blocks are
    a0 = f8(xin+s0)  a1 = f8(xf0+s1)  b0 = f8(xf0+s1 - a1)
    a2 = f8(xf1+s2)  b1 = f8(xf1+s2 - a2)
(s = min-norm solve of W_lin s = b_lin, so the att path needs no bias;
the spill into the val path is folded into bval = b_last - s[128:]@W_last.T.)

Weights are scaled x16 before fp8 quantization (their raw scale ~0.05
sits at e4m3's subnormal floor; x16 lifts weight residuals into normal
range).  PSUM therefore holds 16x pre-activations; the ACT sigmoid uses
scale=1/16 and the final output copy multiplies by 1/16.  Weight
residuals V = f8(16W - f8(16W)) get their own DR terms on the val path
(data fp8 + weight fp8 + both residual corrections = better than bf16:
val rel err ~1.3e-3 vs bf16's 2.4e-3).

Per 128-node subtile the PE does 5 DR matmuls + 1 bf16 seg matmul
(psum bank [128,512]: att cols 0:256, val cols 256:512):
    bias DR: ones(2^-9) @ [B1;B2]        val, start  (B1=f8(64 bval),
             B2=f8(64bval-B1): residual-encoded bias -- a plain fp8 bias
             is a per-column constant whose error accumulates coherently
             over ~30 nodes/graph in the segment sum, +1.3e-2 end-to-end)
    mm1 DR: (a0,a1) @ [Ul0;Ul1]          att, start
    mm3 DR: (a1,b0) @ [U0;U0]            val
    mm4 DR: (a1,a2) @ [V0;V1]            val (weight residuals)
    mm2 DR: (a2,b1) @ [[Ul2|U1];[Ul2|U1]] att+val, stop (b1 rides free
             as att data-residual for xf1)
    ACT: att = sigmoid(psum_att / 16)    -> bf16
    Pool: oh = (iota == idx[:,s]) bf16   (one-hot built on device from a
          2B/node index stream instead of a 256B/node one-hot DMA)
    DVE: g = att * psum_val              -> bf16 (16x scaled, exact shift)
    PE : oh.T @ g += seg_psum[128 graphs, 256]   (lags SEGLAG subtiles)
Window end: ACT copies seg psum * (1/16) -> out DMA.

PE cost: 5 DR mms (4x128c + 1x256c) + seg 256c = 1024 cycles/subtile
(vs 1408 for the previous bf16/fp8-mix kernel); DMA 642B/node
(vs 1024).  End-to-end numpy-sim rel err ~1.35e-2 (gate 2e-2).
"""

import numpy as np
import ml_dtypes

import concourse.bacc as bacc
import concourse.tile as tile
from concourse import mybir
from concourse import bass_utils
from concourse._compat import with_exitstack

P = 128
HID = 256
WIN_SUB = 29                     # subtiles (128 nodes) per window
WIN_NODES = WIN_SUB * P          # 3712
WINDOWS_PER_CORE = 17
N_CORES = 8
NUM_GRAPHS = 16384
GMAX = P                         # graph span per window
SEGLAG = 3                       # seg MM trails the body by this many subtiles
NBLK = 5                         # fp8 blocks per subtile: a0 a1 b0 a2 b1
WSCALE = 16.0                    # weight scale in psum

BF16 = mybir.dt.bfloat16
F32 = mybir.dt.float32
FP8 = mybir.dt.float8e4
npbf16 = ml_dtypes.bfloat16
npf8 = ml_dtypes.float8_e4m3

CHUNKS0 = [2, 6, WIN_SUB - 8]    # window-0 DMA chunking (subtiles)
# fp8 const columns (per DR half): Ul01(256) U00(256) V01(256) A2(512)
# Bias(256) ones(128)
CW_UL01, CW_U00, CW_V01, CW_A2, CW_BIAS, CW_ONES = 0, 256, 512, 768, 1280, 1536
CONST_W8 = 1664


# ----------------------------------------------------------------------------
# host-side planning
# ----------------------------------------------------------------------------

def _build_windows(gi: np.ndarray, num_graphs: int):
    """Greedy windows: contiguous whole-graph ranges, graph span <= GMAX,
    node count <= WIN_NODES.  Returns list of (gbase, gcnt, nstart, ncnt)."""
    counts = np.bincount(gi, minlength=num_graphs)
    starts = np.concatenate([[0], np.cumsum(counts)])
    wins = []
    g = 0
    while g < num_graphs:
        base = g
        nodes = 0
        cnt = 0
        while g < num_graphs and cnt < GMAX and nodes + counts[g] <= WIN_NODES:
            nodes += int(counts[g])
            cnt += 1
            g += 1
        if cnt == 0:
            raise ValueError(f"graph {g} has {counts[g]} nodes > {WIN_NODES}")
        wins.append((base, cnt, int(starts[base]), nodes))
    return wins


# ----------------------------------------------------------------------------
# device kernel
# ----------------------------------------------------------------------------

@with_exitstack
def _device_kernel(ctx, tc, out_ap, ins, n_windows):
    nc = tc.nc
    x_ap, idx_ap, w8_ap, wbf_ap = ins

    consts = ctx.enter_context(tc.tile_pool(name="consts", bufs=1))
    xpool = ctx.enter_context(tc.tile_pool(name="x", bufs=2))
    x0pool = ctx.enter_context(tc.tile_pool(name="x0", bufs=1))
    idxpool = ctx.enter_context(tc.tile_pool(name="idx", bufs=2))
    apool = ctx.enter_context(tc.tile_pool(name="act", bufs=6))
    gpool = ctx.enter_context(tc.tile_pool(name="g", bufs=6))
    ohpool = ctx.enter_context(tc.tile_pool(name="oh", bufs=6))
    outpool = ctx.enter_context(tc.tile_pool(name="out", bufs=2))
    ps_sub = ctx.enter_context(tc.tile_pool(name="ps_sub", bufs=6, space="PSUM"))
    ps_seg = ctx.enter_context(tc.tile_pool(name="ps_seg", bufs=2, space="PSUM"))

    # fp8 consts, one DMA: [128, 2, CONST_W8]
    w8 = consts.tile([P, 2, CONST_W8], FP8)
    w_ul01 = w8[:, :, CW_UL01:CW_UL01 + 256]
    w_u00 = w8[:, :, CW_U00:CW_U00 + 256]
    w_v01 = w8[:, :, CW_V01:CW_V01 + 256]
    w_a2 = w8[:, :, CW_A2:CW_A2 + 512]
    w_bias = w8[:, :, CW_BIAS:CW_BIAS + 256]
    ones9 = w8[:, :, CW_ONES:CW_ONES + P]
    # bf16 consts: iota row 0..127 on every partition
    wbf = consts.tile([P, P], BF16)

    def load_consts():
        nc.scalar.dma_start(w8[:], w8_ap[:])
        nc.scalar.dma_start(wbf[:], wbf_ap[:])

    n_sub = n_windows * WIN_SUB

    # per-window input: one fp8 DMA (chunked for window 0) + one idx DMA
    x_t = [None] * n_windows      # [(tile, subtile offset within tile)]
    idx_t = [None] * n_windows

    def load_window(w):
        it = idxpool.tile([P, WIN_SUB], F32, tag="idx")
        nc.scalar.dma_start(it[:], idx_ap[:, w * WIN_SUB:(w + 1) * WIN_SUB])
        idx_t[w] = it
        span = NBLK * P
        if w == 0:
            x_t[w] = []
            c0 = 0
            for q, csub in enumerate(CHUNKS0):
                t = x0pool.tile([P, csub, NBLK, P], FP8, tag=f"xc{q}")
                nc.sync.dma_start(
                    t[:], x_ap[:, c0 * span:(c0 + csub) * span])
                x_t[w] += [(t, s - c0) for s in range(c0, c0 + csub)]
                c0 += csub
                if q == 0:
                    load_consts()
        else:
            t = xpool.tile([P, WIN_SUB, NBLK, P], FP8, tag="xw")
            base = w * WIN_SUB * span
            nc.sync.dma_start(
                t[:], x_ap[:, base:base + WIN_SUB * span])
            x_t[w] = [(t, s) for s in range(WIN_SUB)]

    seg_tiles = [None] * n_windows
    g_tiles = {}

    DR = mybir.MatmulPerfMode.DoubleRow

    def emit_body(w, s):
        ps = ps_sub.tile([P, 2 * HID], F32, tag="ps")
        xt, xs = x_t[w][s]
        # block order per subtile: a0 a1 b0 a2 b1.  One start (the 512-wide
        # mm) and one stop per bank: the sim tracks psum groups per region.
        nc.tensor.matmul(ps[:, 0:2 * HID], lhsT=xt[:, xs, 3:5, :], rhs=w_a2,
                         start=True, stop=False, perf_mode=DR)
        nc.tensor.matmul(ps[:, 0:HID], lhsT=xt[:, xs, 0:2, :], rhs=w_ul01,
                         start=False, stop=False, perf_mode=DR)
        nc.tensor.matmul(ps[:, HID:2 * HID], lhsT=ones9, rhs=w_bias,
                         start=False, stop=False, perf_mode=DR)
        nc.tensor.matmul(ps[:, HID:2 * HID], lhsT=xt[:, xs, 1:3, :], rhs=w_u00,
                         start=False, stop=False, perf_mode=DR)
        nc.tensor.matmul(ps[:, HID:2 * HID], lhsT=xt[:, xs, 1:4:2, :], rhs=w_v01,
                         start=False, stop=True, perf_mode=DR)
        att = apool.tile([P, HID], BF16, tag="att")
        nc.scalar.activation(att[:], ps[:, 0:HID],
                             mybir.ActivationFunctionType.Sigmoid,
                             scale=1.0 / WSCALE)
        g_sb = gpool.tile([P, HID], BF16, tag="g")
        nc.vector.tensor_tensor(g_sb[:], att[:], ps[:, HID:2 * HID],
                                op=mybir.AluOpType.mult)
        g_tiles[(w, s)] = g_sb

    def emit_seg(w, s):
        if s == 0:
            seg_tiles[w] = ps_seg.tile([P, HID], F32, tag="seg", name="seg")
        seg = seg_tiles[w]
        oh = ohpool.tile([P, P], BF16, tag="oh")
        nc.gpsimd.tensor_scalar(oh[:], wbf[:], idx_t[w][:, s:s + 1], None,
                                op0=mybir.AluOpType.is_equal)
        g_sb = g_tiles.pop((w, s))
        nc.tensor.matmul(seg[:, :], lhsT=oh[:], rhs=g_sb[:],
                         start=(s == 0), stop=(s == WIN_SUB - 1))
        if s == WIN_SUB - 1:
            out_t = outpool.tile([P, HID], F32)
            nc.scalar.activation(out_t[:], seg[:, :],
                                 mybir.ActivationFunctionType.Copy,
                                 scale=1.0 / WSCALE)
            nc.sync.dma_start(out_ap[w * P:(w + 1) * P, :], out_t[:])

    load_window(0)
    for t in range(n_sub):
        w, s = divmod(t, WIN_SUB)
        if s == 0 and w + 1 < n_windows:
            load_window(w + 1)
        emit_body(w, s)
        if t >= SEGLAG:
            emit_seg(*divmod(t - SEGLAG, WIN_SUB))
    for t in range(n_sub - SEGLAG, n_sub):
        emit_seg(*divmod(t, WIN_SUB))


def build_module(n_windows=WINDOWS_PER_CORE):
    nc = bacc.Bacc("TRN2", debug=False, num_devices=N_CORES)
    nn = n_windows * WIN_NODES
    ins = [
        nc.dram_tensor("x", [P, NBLK * nn], FP8, kind="ExternalInput").ap(),
        nc.dram_tensor("idx", [P, n_windows * WIN_SUB], F32,
                       kind="ExternalInput").ap(),
        nc.dram_tensor("w8", [P, 2, CONST_W8], FP8, kind="ExternalInput").ap(),
        nc.dram_tensor("wbf", [P, P], BF16, kind="ExternalInput").ap(),
    ]
    out_ap = nc.dram_tensor("out", [n_windows * P, HID], F32,
                            kind="ExternalOutput").ap()
    with tile.TileContext(nc) as tc:
        _device_kernel(tc, out_ap, ins, n_windows)
    nc.compile()
    return nc


# ----------------------------------------------------------------------------
# host-side data prep
# ----------------------------------------------------------------------------

def _f8(a):
    return np.clip(a, -240.0, 240.0).astype(npf8)


def _prep(inputs, n_windows):
    gi = np.asarray(inputs["graph_index"]).astype(np.int64)
    x_in = np.asarray(inputs["input_rep"], dtype=np.float32)
    x_fin = np.asarray(inputs["final_rep"], dtype=np.float32)
    W_lin = np.asarray(inputs["W_lin"], dtype=np.float64)
    b_lin = np.asarray(inputs["b_lin"], dtype=np.float64)
    W_last = np.asarray(inputs["W_last"], dtype=np.float64)
    b_last = np.asarray(inputs["b_last"], dtype=np.float64)

    if np.any(np.diff(gi) < 0):
        order = np.argsort(gi, kind="stable")
        gi = gi[order]
        x_in = x_in[order]
        x_fin = x_fin[order]

    wins = _build_windows(gi, NUM_GRAPHS)
    budget = N_CORES * n_windows
    assert len(wins) <= budget, f"{len(wins)} windows > budget {budget}"
    wins = wins + [(NUM_GRAPHS, 0, len(gi), 0)] * (budget - len(wins))

    # fold b_lin into the node features: min-norm s with W_lin @ s = b_lin
    s_shift = np.linalg.lstsq(W_lin, b_lin, rcond=None)[0]      # [384]
    bval = b_last - s_shift[128:] @ W_last.T                     # [256]
    s32 = s_shift.astype(np.float32)

    # fp8 data blocks (a = f8(x+s), b = f8 residual), [N, 128] each
    xin_s = x_in + s32[None, :128]
    xf0_s = x_fin[:, 0:P] + s32[None, 128:256]
    xf1_s = x_fin[:, P:2 * P] + s32[None, 256:384]
    a0 = _f8(xin_s)
    a1 = _f8(xf0_s)
    b0 = _f8(xf0_s - a1.astype(np.float32))
    a2 = _f8(xf1_s)
    b1 = _f8(xf1_s - a2.astype(np.float32))
    blocks = [a0, a1, b0, a2, b1]

    # fp8 weights at WSCALE, plus residuals
    WlinT = W_lin.T * WSCALE               # [384, 256] f64, x16
    WlastT = W_last.T * WSCALE             # [256, 256]
    Ul = [_f8(WlinT[i * P:(i + 1) * P]) for i in range(3)]
    U = [_f8(WlastT[i * P:(i + 1) * P]) for i in range(2)]
    V = [_f8(WlastT[i * P:(i + 1) * P] - U[i].astype(np.float64))
         for i in range(2)]

    w8 = np.zeros((P, 2, CONST_W8), npf8)
    w8[:, 0, CW_UL01:CW_UL01 + 256] = Ul[0]
    w8[:, 1, CW_UL01:CW_UL01 + 256] = Ul[1]
    w8[:, 0, CW_U00:CW_U00 + 256] = U[0]
    w8[:, 1, CW_U00:CW_U00 + 256] = U[0]
    w8[:, 0, CW_V01:CW_V01 + 256] = V[0]
    w8[:, 1, CW_V01:CW_V01 + 256] = V[1]
    w8[:, 0, CW_A2:CW_A2 + 256] = Ul[2]
    w8[:, 0, CW_A2 + 256:CW_A2 + 512] = U[1]
    w8[:, 1, CW_A2:CW_A2 + 256] = Ul[2]
    w8[:, 1, CW_A2 + 256:CW_A2 + 512] = U[1]
    # bias: psum += 128 * 2^-9 * (B1 + B2) = 0.25*(B1+B2) ~= 16*bval
    B1 = _f8(4 * WSCALE * bval)
    B2 = _f8(4 * WSCALE * bval - B1.astype(np.float64))
    w8[:, 0, CW_BIAS:CW_BIAS + 256] = np.tile(B1[None, :], (P, 1))
    w8[:, 1, CW_BIAS:CW_BIAS + 256] = np.tile(B2[None, :], (P, 1))
    w8[:, :, CW_ONES:CW_ONES + P] = np.float32(2.0 ** -9)

    wbf = np.tile(np.arange(P, dtype=np.float32)[None, :],
                  (P, 1)).astype(npbf16)

    nn = n_windows * WIN_NODES
    in_maps = []
    for c in range(N_CORES):
        x8 = np.zeros((P, n_windows, WIN_SUB, NBLK, P), npf8)
        idxs = np.full((P, n_windows * WIN_SUB), 255.0, np.float32)
        for j in range(n_windows):
            gb, gc, ns, ncnt = wins[c * n_windows + j]
            if ncnt > 0:
                nspan = min(ncnt, WIN_NODES)
                # [ncnt,128] -> [128, WIN_SUB(partial), 128] feature-major
                for k, src in enumerate(blocks):
                    blk = np.zeros((WIN_NODES, P), npf8)
                    blk[:nspan] = src[ns:ns + nspan]
                    x8[:, j, :, k, :] = blk.reshape(
                        WIN_SUB, P, P).transpose(2, 0, 1)
                loc = np.full((WIN_NODES,), 255, np.int32)
                loc[:nspan] = (gi[ns:ns + nspan] - gb).astype(np.int32)
                idxs[:, j * WIN_SUB:(j + 1) * WIN_SUB] = (
                    loc.reshape(WIN_SUB, P).T.astype(np.float32))
        in_maps.append({
            "x": x8.reshape(P, NBLK * nn),
            "idx": idxs,
            "w8": w8,
            "wbf": wbf,
        })
    return wins, in_maps


def _assemble(wins, results, n_windows):
    out = np.zeros((NUM_GRAPHS, HID), np.float32)
    for c in range(N_CORES):
        res = results[c]["out"]
        for j in range(n_windows):
            gb, gc, _, _ = wins[c * n_windows + j]
            if gc == 0:
                continue
            out[gb:gb + gc] = res[j * P:j * P + gc]
    return out


# ----------------------------------------------------------------------------
# entry point
# ----------------------------------------------------------------------------

_CACHE = {}
LAST_RESULTS = None


def kernel(**inputs) -> np.ndarray:
    global LAST_RESULTS
    gi = np.asarray(inputs["graph_index"]).astype(np.int64)
    n_wins_needed = len(_build_windows(np.sort(gi), NUM_GRAPHS))
    n_windows = max(WINDOWS_PER_CORE, -(-n_wins_needed // N_CORES))
    if n_windows not in _CACHE:
        _CACHE[n_windows] = build_module(n_windows)
    nc = _CACHE[n_windows]
    wins, in_maps = _prep(inputs, n_windows)
    # a previously-wedged core can fail one run with
    # NRT_EXEC_UNIT_UNRECOVERABLE and reset itself; retry once
    try:
        res = bass_utils.run_bass_kernel_spmd(
            nc, in_maps, core_ids=list(range(N_CORES)))
    except Exception:
        res = bass_utils.run_bass_kernel_spmd(
            nc, in_maps, core_ids=list(range(N_CORES)))
    LAST_RESULTS = res
    return _assemble(wins, res.results, n_windows)


# revision 7
# speedup vs baseline: 3.0487x; 3.0487x over previous
"""AttentionPooling (segment_reduce) Trainium2 kernel.

att = sigmoid([input_rep, final_rep] @ W_lin.T + b_lin)
g   = att * (final_rep @ W_last.T + b_last)
out = segment_sum(g, graph_index, 16384)          # graph_index sorted

Strategy (8 NeuronCores, pure data-parallel, no collectives):
  graph_index is sorted, so a contiguous node range covers a contiguous
  graph range.  Host greedily packs whole graphs into "windows" of
  <= WIN_NODES nodes spanning <= 128 graphs; ~137 windows cover all 500k
  nodes = 8 cores x 17 windows.  Each core gets its windows as a padded
  node stream in feature-major layout: fp8 blocks [a0=f8(xin+s0),
  a1=f8(xf0+s1), onehot] + bf16 blocks [xf0+s1, xf1+s2].

Measured PE model (microbenched): every matmul streams out_cols x 1
cycle @2.4GHz regardless of dtype; fp8 DoubleRow packs K=256 into one
mm at no extra cost (=2x MACs/cycle).  So the kernel minimizes total
output columns per subtile; fp8 is used where a K=256 DR pack saves a
chunk pass, bf16 where accuracy needs it (val path).

b_lin is folded into the inputs (min-norm s with W_lin@s = b_lin; the
spill into the val path moves into bval = b_last - s[128:]@W_last.T).
Att fp8 weights are scaled x16 (lifts small weights out of e4m3's
subnormal floor); the ACT sigmoid descales via scale=1/16.  The shared
mm1 rhs is [16*Wlin2 | Wlast1] so its att half matches the x16 scale
while its val half stays x1.

Per 128-node subtile (psum bank [128,512]: att cols 0:256, val 256:512):
    mm1: xf1b @ [16*Wlin2|Wlast1]  bf16, 0:512, start  (512c)
    mm2: DR (a0,a1) @ [16Wlin0;16Wlin1] fp8, att       (256c)
    mm3: xf0b @ Wlast0             bf16, val           (256c)
    bias (ACT_BIAS=False): ones @ bval/128 bf16, val, stop  (256c)
         (ACT_BIAS=True): ACT seeds bval f32 into val cols before mm1;
         mm order flips so mm2 starts the att range instead.
    ACT: att = sigmoid(psum_att / 16)  -> bf16
    DVE: g = att * psum_val            -> bf16
    PE : oh.T @ g += seg_psum          (fp8 one-hot, lags SEGLAG)
Window end: ACT copies seg psum -> out DMA.

PE: 1536 cycles/subtile (1280 with ACT_BIAS) vs 1792 for the previous
kernel; DMA 896B/node vs 1024.  Expected rel err ~1.2e-2 (gate 2e-2).
"""

import numpy as np
import ml_dtypes

import concourse.bacc as bacc
import concourse.tile as tile
from concourse import mybir
from concourse import bass_utils
from concourse._compat import with_exitstack

P = 128
HID = 256
WIN_SUB = 29                     # subtiles (128 nodes) per window
WIN_NODES = WIN_SUB * P          # 3712
WINDOWS_PER_CORE = 17
N_CORES = 8
NUM_GRAPHS = 16384
GMAX = P                         # graph span per window
SEGLAG = 3                       # seg MM trails the body by this many subtiles
WSCALE = 16.0                    # att fp8 weight scale
# bias via ACT psum write is impossible: matmul start_tensor_calc zeroes the
# whole 2KB psum bank (ZERO_REGION_SIZE), wiping any engine-seeded content.
ACT_BIAS = False

BF16 = mybir.dt.bfloat16
F32 = mybir.dt.float32
FP8 = mybir.dt.float8e4
npbf16 = ml_dtypes.bfloat16
npf8 = ml_dtypes.float8_e4m3

CHUNKS0 = [2, 6, WIN_SUB - 8]    # window-0 DMA chunking (subtiles)


def _build_windows(gi: np.ndarray, num_graphs: int):
    """Greedy windows: contiguous whole-graph ranges, graph span <= GMAX,
    node count <= WIN_NODES.  Returns list of (gbase, gcnt, nstart, ncnt)."""
    counts = np.bincount(gi, minlength=num_graphs)
    starts = np.concatenate([[0], np.cumsum(counts)])
    wins = []
    g = 0
    while g < num_graphs:
        base = g
        nodes = 0
        cnt = 0
        while g < num_graphs and cnt < GMAX and nodes + counts[g] <= WIN_NODES:
            nodes += int(counts[g])
            cnt += 1
            g += 1
        if cnt == 0:
            raise ValueError(f"graph {g} has {counts[g]} nodes > {WIN_NODES}")
        wins.append((base, cnt, int(starts[base]), nodes))
    return wins


# ----------------------------------------------------------------------------
# device kernel
# ----------------------------------------------------------------------------

@with_exitstack
def _device_kernel(ctx, tc, out_ap, ins, n_windows):
    nc = tc.nc
    x8_ap, xb_ap, w8_ap, wb_ap, bias_ap = ins

    consts = ctx.enter_context(tc.tile_pool(name="consts", bufs=1))
    x8pool = ctx.enter_context(tc.tile_pool(name="x8", bufs=2))
    xbpool = ctx.enter_context(tc.tile_pool(name="xb", bufs=2))
    x80pool = ctx.enter_context(tc.tile_pool(name="x80", bufs=1))
    xb0pool = ctx.enter_context(tc.tile_pool(name="xb0", bufs=1))
    apool = ctx.enter_context(tc.tile_pool(name="act", bufs=6))
    gpool = ctx.enter_context(tc.tile_pool(name="g", bufs=6))
    outpool = ctx.enter_context(tc.tile_pool(name="out", bufs=2))
    ps_sub = ctx.enter_context(tc.tile_pool(name="ps_sub", bufs=6, space="PSUM"))
    ps_seg = ctx.enter_context(tc.tile_pool(name="ps_seg", bufs=2, space="PSUM"))

    # consts: fp8 DR weights [128,2,256], bf16 [wcat1(512)|wlast0(256)|
    # ones(128)|bvalmm(256)], f32 bias [128,256]
    w8 = consts.tile([P, 2, HID], FP8)
    wb = consts.tile([P, 512 + 256 + 128 + 256], BF16)
    wcat1 = wb[:, 0:512]
    wlast0 = wb[:, 512:768]
    ones_t = wb[:, 768:896]
    bvalmm = wb[:, 896:1152]
    biasf = consts.tile([P, HID], F32)

    def load_consts():
        nc.scalar.dma_start(w8[:], w8_ap[:])
        nc.scalar.dma_start(wb[:], wb_ap[:])
        nc.scalar.dma_start(biasf[:], bias_ap[:])

    n_sub = n_windows * WIN_SUB
    x8_t = [None] * n_windows     # [(tile, subtile offset)]
    xb_t = [None] * n_windows

    def load_window(w):
        if w == 0:
            x8_t[w], xb_t[w] = [], []
            c0 = 0
            for q, csub in enumerate(CHUNKS0):
                t8 = x80pool.tile([P, csub, 3, P], FP8, tag=f"x8c{q}")
                nc.sync.dma_start(
                    t8[:], x8_ap[:, c0 * 3 * P:(c0 + csub) * 3 * P])
                tb = xb0pool.tile([P, csub, 2, P], BF16, tag=f"xbc{q}")
                nc.sync.dma_start(
                    tb[:], xb_ap[:, c0 * 2 * P:(c0 + csub) * 2 * P])
                x8_t[w] += [(t8, s - c0) for s in range(c0, c0 + csub)]
                xb_t[w] += [(tb, s - c0) for s in range(c0, c0 + csub)]
                c0 += csub
                if q == 0:
                    load_consts()
        else:
            t8 = x8pool.tile([P, WIN_SUB, 3, P], FP8, tag="x8w")
            nc.sync.dma_start(
                t8[:], x8_ap[:, (w * WIN_SUB) * 3 * P:((w + 1) * WIN_SUB) * 3 * P])
            tb = xbpool.tile([P, WIN_SUB, 2, P], BF16, tag="xbw")
            nc.sync.dma_start(
                tb[:], xb_ap[:, (w * WIN_SUB) * 2 * P:((w + 1) * WIN_SUB) * 2 * P])
            x8_t[w] = [(t8, s) for s in range(WIN_SUB)]
            xb_t[w] = [(tb, s) for s in range(WIN_SUB)]

    seg_tiles = [None] * n_windows
    g_tiles = {}
    DRM = mybir.MatmulPerfMode.DoubleRow

    def emit_body(w, s):
        ps = ps_sub.tile([P, 2 * HID], F32, tag="ps")
        t8, s8 = x8_t[w][s]
        tb, sb = xb_t[w][s]
        if ACT_BIAS:
            # seed val cols with the f32 bias, then accumulate everything
            nc.scalar.copy(ps[:, HID:2 * HID], biasf[:])
            nc.tensor.matmul(ps[:, 0:HID], lhsT=t8[:, s8, 0:2, :], rhs=w8[:],
                             start=True, stop=False, perf_mode=DRM)
            nc.tensor.matmul(ps[:, 0:2 * HID], lhsT=tb[:, sb, 1, :], rhs=wcat1,
                             start=False, stop=False, skip_group_check=True)
            nc.tensor.matmul(ps[:, HID:2 * HID], lhsT=tb[:, sb, 0, :],
                             rhs=wlast0, start=False, stop=True,
                             skip_group_check=True)
        else:
            nc.tensor.matmul(ps[:, 0:2 * HID], lhsT=tb[:, sb, 1, :], rhs=wcat1,
                             start=True, stop=False)
            nc.tensor.matmul(ps[:, 0:HID], lhsT=t8[:, s8, 0:2, :], rhs=w8[:],
                             start=False, stop=False, perf_mode=DRM)
            nc.tensor.matmul(ps[:, HID:2 * HID], lhsT=tb[:, sb, 0, :],
                             rhs=wlast0, start=False, stop=False)
            nc.tensor.matmul(ps[:, HID:2 * HID], lhsT=ones_t, rhs=bvalmm,
                             start=False, stop=True)
        att = apool.tile([P, HID], BF16, tag="att")
        nc.scalar.activation(att[:], ps[:, 0:HID],
                             mybir.ActivationFunctionType.Sigmoid,
                             scale=1.0 / WSCALE)
        g_sb = gpool.tile([P, HID], BF16, tag="g")
        nc.vector.tensor_tensor(g_sb[:], att[:], ps[:, HID:2 * HID],
                                op=mybir.AluOpType.mult)
        g_tiles[(w, s)] = g_sb

    def emit_seg(w, s):
        if s == 0:
            seg_tiles[w] = ps_seg.tile([P, HID], F32, tag="seg", name="seg")
        seg = seg_tiles[w]
        t8, s8 = x8_t[w][s]
        g_sb = g_tiles.pop((w, s))
        nc.tensor.matmul(seg[:, :], lhsT=t8[:, s8, 2, :], rhs=g_sb[:],
                         start=(s == 0), stop=(s == WIN_SUB - 1))
        if s == WIN_SUB - 1:
            out_t = outpool.tile([P, HID], F32)
            nc.scalar.copy(out_t[:], seg[:, :])
            nc.sync.dma_start(out_ap[w * P:(w + 1) * P, :], out_t[:])

    load_window(0)
    for t in range(n_sub):
        w, s = divmod(t, WIN_SUB)
        if s == 0 and w + 1 < n_windows:
            load_window(w + 1)
        emit_body(w, s)
        if t >= SEGLAG:
            emit_seg(*divmod(t - SEGLAG, WIN_SUB))
    for t in range(n_sub - SEGLAG, n_sub):
        emit_seg(*divmod(t, WIN_SUB))


def build_module(n_windows=WINDOWS_PER_CORE):
    nc = bacc.Bacc("TRN2", debug=False, num_devices=N_CORES)
    nn = n_windows * WIN_NODES
    ins = [
        nc.dram_tensor("x8", [P, 3 * nn], FP8, kind="ExternalInput").ap(),
        nc.dram_tensor("xb", [P, 2 * nn], BF16, kind="ExternalInput").ap(),
        nc.dram_tensor("w8", [P, 2, HID], FP8, kind="ExternalInput").ap(),
        nc.dram_tensor("wb", [P, 1152], BF16, kind="ExternalInput").ap(),
        nc.dram_tensor("biasf", [P, HID], F32, kind="ExternalInput").ap(),
    ]
    out_ap = nc.dram_tensor("out", [n_windows * P, HID], F32,
                            kind="ExternalOutput").ap()
    with tile.TileContext(nc) as tc:
        _device_kernel(tc, out_ap, ins, n_windows)
    nc.compile()
    return nc


# ----------------------------------------------------------------------------
# host-side data prep
# ----------------------------------------------------------------------------

def _f8(a):
    return np.clip(a, -240.0, 240.0).astype(npf8)


def _prep(inputs, n_windows):
    gi = np.asarray(inputs["graph_index"]).astype(np.int64)
    x_in = np.asarray(inputs["input_rep"], dtype=np.float32)
    x_fin = np.asarray(inputs["final_rep"], dtype=np.float32)
    W_lin = np.asarray(inputs["W_lin"], dtype=np.float64)
    b_lin = np.asarray(inputs["b_lin"], dtype=np.float64)
    W_last = np.asarray(inputs["W_last"], dtype=np.float64)
    b_last = np.asarray(inputs["b_last"], dtype=np.float64)

    if np.any(np.diff(gi) < 0):
        order = np.argsort(gi, kind="stable")
        gi = gi[order]
        x_in = x_in[order]
        x_fin = x_fin[order]

    wins = _build_windows(gi, NUM_GRAPHS)
    budget = N_CORES * n_windows
    assert len(wins) <= budget, f"{len(wins)} windows > budget {budget}"
    wins = wins + [(NUM_GRAPHS, 0, len(gi), 0)] * (budget - len(wins))

    # fold b_lin into the node features: min-norm s with W_lin @ s = b_lin
    s_shift = np.linalg.lstsq(W_lin, b_lin, rcond=None)[0]      # [384]
    bval = b_last - s_shift[128:] @ W_last.T                     # [256]
    s32 = s_shift.astype(np.float32)

    xin_s = x_in + s32[None, :128]
    xf0_s = x_fin[:, 0:P] + s32[None, 128:256]
    xf1_s = x_fin[:, P:2 * P] + s32[None, 256:384]
    a0 = _f8(xin_s)
    a1 = _f8(xf0_s)
    xf0_b = xf0_s.astype(npbf16)
    xf1_b = xf1_s.astype(npbf16)

    WlinT = W_lin.T                   # [384, 256] f64
    WlastT = W_last.T                 # [256, 256]
    w8 = np.zeros((P, 2, HID), npf8)
    w8[:, 0, :] = _f8(WSCALE * WlinT[0:P])
    w8[:, 1, :] = _f8(WSCALE * WlinT[P:2 * P])
    wb = np.zeros((P, 1152), npbf16)
    wb[:, 0:256] = (WSCALE * WlinT[2 * P:3 * P]).astype(npbf16)
    wb[:, 256:512] = WlastT[P:2 * P].astype(npbf16)
    wb[:, 512:768] = WlastT[0:P].astype(npbf16)
    wb[:, 768:896] = np.ones((P, P), npbf16)
    wb[:, 896:1152] = np.tile((bval / P)[None, :], (P, 1)).astype(npbf16)
    biasf = np.tile(bval[None, :].astype(np.float32), (P, 1))

    nn = n_windows * WIN_NODES
    jgrid = np.arange(P, dtype=np.int32)
    in_maps = []
    for c in range(N_CORES):
        x8 = np.zeros((P, n_windows, WIN_SUB, 3, P), npf8)
        xb = np.zeros((P, n_windows, WIN_SUB, 2, P), npbf16)
        for j in range(n_windows):
            gb, gc, ns, ncnt = wins[c * n_windows + j]
            if ncnt > 0:
                for k, src in enumerate((a0, a1)):
                    blk = np.zeros((WIN_NODES, P), npf8)
                    blk[:ncnt] = src[ns:ns + ncnt]
                    x8[:, j, :, k, :] = blk.reshape(
                        WIN_SUB, P, P).transpose(2, 0, 1)
                for k, src in enumerate((xf0_b, xf1_b)):
                    blk = np.zeros((WIN_NODES, P), npbf16)
                    blk[:ncnt] = src[ns:ns + ncnt]
                    xb[:, j, :, k, :] = blk.reshape(
                        WIN_SUB, P, P).transpose(2, 0, 1)
                # one-hot fp8: oh[feat=graphslot j2, node] = (loc == j2)
                loc = np.full((WIN_NODES,), -1, np.int32)
                loc[:ncnt] = (gi[ns:ns + ncnt] - gb).astype(np.int32)
                a = loc.reshape(WIN_SUB, P)                      # [s, n]
                ohw = (a[:, :, None] == jgrid[None, None, :])    # [s, n, j]
                x8[:, j, :, 2, :] = ohw.transpose(1, 0, 2).astype(npf8)
        in_maps.append({
            "x8": x8.reshape(P, 3 * nn),
            "xb": xb.reshape(P, 2 * nn),
            "w8": w8, "wb": wb, "biasf": biasf,
        })
    return wins, in_maps


def _assemble(wins, results, n_windows):
    out = np.zeros((NUM_GRAPHS, HID), np.float32)
    for c in range(N_CORES):
        res = results[c]["out"]
        for j in range(n_windows):
            gb, gc, _, _ = wins[c * n_windows + j]
            if gc == 0:
                continue
            out[gb:gb + gc] = res[j * P:j * P + gc]
    return out


# ----------------------------------------------------------------------------
# entry point
# ----------------------------------------------------------------------------

_CACHE = {}
LAST_RESULTS = None


def kernel(**inputs) -> np.ndarray:
    global LAST_RESULTS
    gi = np.asarray(inputs["graph_index"]).astype(np.int64)
    n_wins_needed = len(_build_windows(np.sort(gi), NUM_GRAPHS))
    n_windows = max(WINDOWS_PER_CORE, -(-n_wins_needed // N_CORES))
    if n_windows not in _CACHE:
        _CACHE[n_windows] = build_module(n_windows)
    nc = _CACHE[n_windows]
    wins, in_maps = _prep(inputs, n_windows)
    # a previously-wedged core can fail one run with
    # NRT_EXEC_UNIT_UNRECOVERABLE and reset itself; retry once
    try:
        res = bass_utils.run_bass_kernel_spmd(
            nc, in_maps, core_ids=list(range(N_CORES)))
    except Exception:
        res = bass_utils.run_bass_kernel_spmd(
            nc, in_maps, core_ids=list(range(N_CORES)))
    LAST_RESULTS = res
    return _assemble(wins, res.results, n_windows)
